# revision 2
# baseline (speedup 1.0000x reference)
"""Trainium2 Bass kernel for nn_DeletionLayer: out = where(mask, x @ W, x).

x: [200000, 1024] f32, deletion_weight: [1024, 1024] f32, mask: [200000] bool.

Sharding: data-parallel over the node axis across 8 NeuronCores. Each core
gets a uniform 25088-row (196 x 128) shard; core 7's shard overlaps core 6's
by 704 rows (identical rows recomputed, dropped at gather) so every core runs
the same program with full 128-row tiles only.

Per 128-row tile (on device):
  - DMA the f32 x tile (for bit-exact passthrough of unmasked rows) and the
    pretransposed bf16 lhsT blocks.
  - 16 bf16 matmuls (8 K-chunks x 2 PSUM-bank halves) accumulate xw = x @ W
    into PSUM in f32.
  - DVE copy_predicated overwrites masked rows of the x tile with xw (mask
    broadcast along the free dim), then DMA the tile out.

The 196 tiles run in a single hardware For_i loop (14 iterations x 14-tile
unrolled body) so the program stays small (fast compile) while the back-edge
barrier cost stays ~1% of runtime.
"""

from contextlib import ExitStack

import numpy as np

N_FULL = 200000
DIM = 1024
P = 128
KCH = DIM // P  # 8 contraction chunks
NCH = DIM // 512  # 2 PSUM-bank halves
R = 25088  # rows per core (196 full tiles)
T = R // P  # 196
N_CORES = 8
U = 14  # tiles per loop-body unroll


def _build_nc():
    import concourse.bass as bass
    import concourse.tile as tile
    from concourse import bacc, mybir

    n_loop = T // U
    nc = bacc.Bacc("TRN2", target_bir_lowering=False, debug=False)

    x_dram = nc.dram_tensor("x", [R, DIM], mybir.dt.float32, kind="ExternalInput")
    xt_dram = nc.dram_tensor(
        "xt", [T * DIM, P], mybir.dt.bfloat16, kind="ExternalInput"
    )
    w_dram = nc.dram_tensor("w", [DIM, DIM], mybir.dt.bfloat16, kind="ExternalInput")
    m_dram = nc.dram_tensor("mask", [P, T], mybir.dt.uint8, kind="ExternalInput")
    o_dram = nc.dram_tensor("out", [R, DIM], mybir.dt.float32, kind="ExternalOutput")

    with tile.TileContext(nc) as tc:
        with ExitStack() as ctx:
            wpool = ctx.enter_context(tc.tile_pool(name="w", bufs=1))
            xpool = ctx.enter_context(tc.tile_pool(name="x", bufs=3))
            xtpool = ctx.enter_context(tc.tile_pool(name="xt", bufs=3))
            pso_pool = ctx.enter_context(
                tc.tile_pool(name="psO", bufs=3, space="PSUM")
            )

            w_sb = wpool.tile([P, KCH, DIM], mybir.dt.bfloat16)
            nc.sync.dma_start(w_sb[:], w_dram.ap().rearrange("(c p) d -> p c d", p=P))
            m_all = wpool.tile([P, T], mybir.dt.uint8)
            nc.sync.dma_start(m_all[:], m_dram[:])

            def emit_tile(t):
                x_t = xpool.tile([P, DIM], mybir.dt.float32, tag="x")
                nc.sync.dma_start(x_t[:], x_dram[bass.ts(t, P), :])

                xT = xtpool.tile([P, KCH, P], mybir.dt.bfloat16, tag="xT")
                nc.sync.dma_start(
                    xT[:],
                    xt_dram[bass.ts(t, DIM), :].rearrange("(c i) j -> i c j", i=P),
                )

                psO = pso_pool.tile([P, DIM], mybir.dt.float32, tag="psO")
                for n in range(NCH):
                    for k in range(KCH):
                        nc.tensor.matmul(
                            psO[:, n * 512 : (n + 1) * 512],
                            xT[:, k, :],
                            w_sb[:, k, n * 512 : (n + 1) * 512],
                            start=(k == 0),
                            stop=(k == KCH - 1),
                        )

                nc.vector.copy_predicated(
                    x_t[:],
                    m_all[:, bass.ds(t, 1)].broadcast_to([P, DIM]),
                    psO[:],
                )
                nc.sync.dma_start(o_dram[bass.ts(t, P), :], x_t[:])

            with tc.For_i(0, n_loop, 1) as i:
                for j in range(U):
                    emit_tile(i * U + j)

    nc.compile()
    return nc


def _shard_starts(n):
    return [c * R for c in range(N_CORES - 1)] + [n - R]


def _core_map(xs, ms, w_in):
    import ml_dtypes

    # xt[t, c, i, j] = x[t*128 + j, c*128 + i] — lhsT blocks, bf16
    xt = (
        np.ascontiguousarray(xs.reshape(T, P, KCH, P).transpose(0, 2, 3, 1))
        .astype(ml_dtypes.bfloat16)
        .reshape(T * DIM, P)
    )
    return {
        "x": np.ascontiguousarray(xs),
        "xt": xt,
        "w": w_in,
        "mask": np.ascontiguousarray(ms.astype(np.uint8).reshape(T, P).T),
    }


_cached_nc = None


def _prepare(x, deletion_weight, mask):
    import ml_dtypes

    w_in = deletion_weight.astype(ml_dtypes.bfloat16)
    starts = _shard_starts(x.shape[0])
    return [_core_map(x[r0 : r0 + R], mask[r0 : r0 + R], w_in) for r0 in starts]


def _gather(results, n):
    out = np.empty((n, DIM), np.float32)
    for c in range(N_CORES - 1):
        out[c * R : (c + 1) * R] = results[c]["out"]
    out[n - R :] = results[-1]["out"]
    return out


def kernel(x, deletion_weight, mask):
    global _cached_nc

    from concourse import bass_utils

    x = np.asarray(x, dtype=np.float32)
    deletion_weight = np.asarray(deletion_weight, dtype=np.float32)
    mask = np.asarray(mask)
    n = x.shape[0]
    assert n == N_FULL and x.shape[1] == DIM

    if _cached_nc is None:
        _cached_nc = _build_nc()
    nc = _cached_nc

    in_maps = _prepare(x, deletion_weight, mask)
    res = bass_utils.run_bass_kernel_spmd(
        nc, in_maps, core_ids=list(range(N_CORES))
    )
    return _gather(res.results, n)



# revision 5
# speedup vs baseline: 3.2343x; 3.2343x over previous
"""Trainium2 Bass kernel for nn_DeletionLayer: out = where(mask, x @ W, x).

x: [200000, 1024] f32, deletion_weight: [1024, 1024] f32, mask: [200000] bool.

Sharding: data-parallel over the node axis across 8 NeuronCores. Each core
gets a uniform 25088-row shard; core 7's shard overlaps core 6's by 704 rows
(identical rows recomputed, dropped at gather) so every core runs the same
program.

Fast path (deletion_weight is a constant matrix c — true for this module,
W = ones/1000): x @ W == c * rowsum(x) broadcast across columns, so the
kernel never needs the matmul. Per 512-row supertile (bf16 I/O):
  - DMA the bf16 x supertile [128, 4, 1024].
  - DVE tensor_tensor_reduce folds the two 512-column halves and
    accumulates the f32 rowsum s per row.
  - tensor_scalar forms bias = s * (c*m) per row.
  - ACT activation computes out = (1-m)*x + bias: masked rows become c*s,
    unmasked rows are an exact bf16 passthrough.
  - DMA the bf16 supertile out.
Host converts x f32->bf16 on the way in and out bf16->f32 on the way out;
rel error vs the f32 reference is ~1e-3, well inside the 2e-2 gate.

General path (non-constant W, not exercised by the reference inputs): the
original bf16 matmul kernel (pretransposed lhsT blocks, 16 matmuls per
128-row tile accumulating into PSUM, DVE copy_predicated select, f32 I/O).
"""

from contextlib import ExitStack

import numpy as np

N_FULL = 200000
DIM = 1024
P = 128
KCH = DIM // P  # 8 contraction chunks (general path)
NCH = DIM // 512  # 2 PSUM-bank halves (general path)
R = 25088  # rows per core
T = R // P  # 196 tiles of 128 rows
N_CORES = 8
U = 14  # tiles per loop-body unroll (general path)
G = 4  # 128-row tiles per supertile (fast path)
NS = T // G  # 49 supertiles


def _build_nc_fast(ns=NS, debug=False):
    import concourse.bass as bass
    import concourse.tile as tile
    from concourse import bacc, mybir

    NS, T, R = ns, ns * G, ns * G * P  # noqa: shadowing module consts on purpose
    nc = bacc.Bacc("TRN2", target_bir_lowering=False, debug=debug)

    x_dram = nc.dram_tensor("x", [R, DIM], mybir.dt.bfloat16, kind="ExternalInput")
    cm_dram = nc.dram_tensor("cm", [P, T], mybir.dt.float32, kind="ExternalInput")
    sc_dram = nc.dram_tensor("sc", [P, T], mybir.dt.float32, kind="ExternalInput")
    o_dram = nc.dram_tensor("out", [R, DIM], mybir.dt.bfloat16, kind="ExternalOutput")

    with tile.TileContext(nc) as tc:
        with ExitStack() as ctx:
            cpool = ctx.enter_context(tc.tile_pool(name="const", bufs=1))
            xpool = ctx.enter_context(tc.tile_pool(name="x", bufs=3))
            opool = ctx.enter_context(tc.tile_pool(name="o", bufs=3))
            spool = ctx.enter_context(tc.tile_pool(name="s", bufs=3))

            cm_sb = cpool.tile([P, T], mybir.dt.float32)
            nc.sync.dma_start(cm_sb[:], cm_dram[:])
            sc_sb = cpool.tile([P, T], mybir.dt.float32)
            nc.sync.dma_start(sc_sb[:], sc_dram[:])

            for u in range(NS):
                xs = xpool.tile([P, G, DIM], mybir.dt.bfloat16, tag="xs")
                nc.sync.dma_start(
                    xs[:],
                    x_dram[bass.ts(u, G * P), :].rearrange(
                        "(g p) d -> p g d", p=P
                    ),
                )

                os = opool.tile([P, G, DIM], mybir.dt.bfloat16, tag="os")
                s = spool.tile([P, G], mybir.dt.float32, tag="s")
                bias = spool.tile([P, G], mybir.dt.float32, tag="bias")

                nc.vector.reduce_sum(s[:], xs[:], axis=mybir.AxisListType.X)
                for g in range(G):
                    nc.vector.tensor_scalar_mul(
                        bias[:, g : g + 1],
                        s[:, g : g + 1],
                        cm_sb[:, bass.ds(u * G + g, 1)],
                    )
                for g in range(G):
                    nc.scalar.activation(
                        os[:, g, :],
                        xs[:, g, :],
                        mybir.ActivationFunctionType.Identity,
                        bias=bias[:, g : g + 1],
                        scale=sc_sb[:, bass.ds(u * G + g, 1)],
                    )

                nc.sync.dma_start(
                    o_dram[bass.ts(u, G * P), :].rearrange("(g p) d -> p g d", p=P),
                    os[:],
                )

    nc.compile()
    return nc


def _build_nc_general():
    import concourse.bass as bass
    import concourse.tile as tile
    from concourse import bacc, mybir

    n_loop = T // U
    nc = bacc.Bacc("TRN2", target_bir_lowering=False, debug=False)

    x_dram = nc.dram_tensor("x", [R, DIM], mybir.dt.float32, kind="ExternalInput")
    xt_dram = nc.dram_tensor(
        "xt", [T * DIM, P], mybir.dt.bfloat16, kind="ExternalInput"
    )
    w_dram = nc.dram_tensor("w", [DIM, DIM], mybir.dt.bfloat16, kind="ExternalInput")
    m_dram = nc.dram_tensor("mask", [P, T], mybir.dt.uint8, kind="ExternalInput")
    o_dram = nc.dram_tensor("out", [R, DIM], mybir.dt.float32, kind="ExternalOutput")

    with tile.TileContext(nc) as tc:
        with ExitStack() as ctx:
            wpool = ctx.enter_context(tc.tile_pool(name="w", bufs=1))
            xpool = ctx.enter_context(tc.tile_pool(name="x", bufs=3))
            xtpool = ctx.enter_context(tc.tile_pool(name="xt", bufs=3))
            pso_pool = ctx.enter_context(
                tc.tile_pool(name="psO", bufs=3, space="PSUM")
            )

            w_sb = wpool.tile([P, KCH, DIM], mybir.dt.bfloat16)
            nc.sync.dma_start(w_sb[:], w_dram.ap().rearrange("(c p) d -> p c d", p=P))
            m_all = wpool.tile([P, T], mybir.dt.uint8)
            nc.sync.dma_start(m_all[:], m_dram[:])

            def emit_tile(t):
                x_t = xpool.tile([P, DIM], mybir.dt.float32, tag="x")
                nc.sync.dma_start(x_t[:], x_dram[bass.ts(t, P), :])

                xT = xtpool.tile([P, KCH, P], mybir.dt.bfloat16, tag="xT")
                nc.sync.dma_start(
                    xT[:],
                    xt_dram[bass.ts(t, DIM), :].rearrange("(c i) j -> i c j", i=P),
                )

                psO = pso_pool.tile([P, DIM], mybir.dt.float32, tag="psO")
                for n in range(NCH):
                    for k in range(KCH):
                        nc.tensor.matmul(
                            psO[:, n * 512 : (n + 1) * 512],
                            xT[:, k, :],
                            w_sb[:, k, n * 512 : (n + 1) * 512],
                            start=(k == 0),
                            stop=(k == KCH - 1),
                        )

                nc.vector.copy_predicated(
                    x_t[:],
                    m_all[:, bass.ds(t, 1)].broadcast_to([P, DIM]),
                    psO[:],
                )
                nc.sync.dma_start(o_dram[bass.ts(t, P), :], x_t[:])

            with tc.For_i(0, n_loop, 1) as i:
                for j in range(U):
                    emit_tile(i * U + j)

    nc.compile()
    return nc


def _shard_starts(n):
    return [c * R for c in range(N_CORES - 1)] + [n - R]


def _core_map_fast(xs_bf, ms, c):
    # [u, g, p] row order -> per-partition tables [P, T=NS*G]
    mt = (
        ms.astype(np.float32)
        .reshape(NS, G, P)
        .transpose(2, 0, 1)
        .reshape(P, T)
    )
    return {
        "x": xs_bf,
        "cm": np.ascontiguousarray(mt * c),
        "sc": np.ascontiguousarray(1.0 - mt),
    }


def _core_map_general(xs, ms, w_bf):
    import ml_dtypes

    # xt[t, c, i, j] = x[t*128 + j, c*128 + i] — lhsT blocks, bf16
    xt = (
        np.ascontiguousarray(xs.reshape(T, P, KCH, P).transpose(0, 2, 3, 1))
        .astype(ml_dtypes.bfloat16)
        .reshape(T * DIM, P)
    )
    return {
        "x": np.ascontiguousarray(xs),
        "xt": xt,
        "w": w_bf,
        "mask": np.ascontiguousarray(ms.astype(np.uint8).reshape(T, P).T),
    }


_cached_nc_fast = None
_cached_nc_general = None


def _plan(x, deletion_weight, mask):
    """Returns (nc, in_maps, out_dtype_is_bf16)."""
    global _cached_nc_fast, _cached_nc_general
    import ml_dtypes

    n = x.shape[0]
    starts = _shard_starts(n)
    w = np.asarray(deletion_weight, dtype=np.float32)
    c = w.flat[0]
    if np.all(w == c):
        if _cached_nc_fast is None:
            _cached_nc_fast = _build_nc_fast()
        x_bf = np.asarray(x, dtype=np.float32).astype(ml_dtypes.bfloat16)
        in_maps = [
            _core_map_fast(x_bf[r0 : r0 + R], mask[r0 : r0 + R], float(c))
            for r0 in starts
        ]
        return _cached_nc_fast, in_maps, True
    if _cached_nc_general is None:
        _cached_nc_general = _build_nc_general()
    w_bf = w.astype(ml_dtypes.bfloat16)
    in_maps = [
        _core_map_general(x[r0 : r0 + R], mask[r0 : r0 + R], w_bf) for r0 in starts
    ]
    return _cached_nc_general, in_maps, False


def _gather(results, n):
    out = np.empty((n, DIM), np.float32)
    starts = _shard_starts(n)
    for c in range(N_CORES - 1):
        out[starts[c] : starts[c] + R] = results[c]["out"]
    tail = n - R * (N_CORES - 1)
    out[n - tail :] = results[-1]["out"][R - tail :]
    return out


def kernel(x, deletion_weight, mask):
    from concourse import bass_utils

    x = np.asarray(x, dtype=np.float32)
    mask = np.asarray(mask)
    n = x.shape[0]
    assert n == N_FULL and x.shape[1] == DIM

    nc, in_maps, _ = _plan(x, deletion_weight, mask)
    res = bass_utils.run_bass_kernel_spmd(nc, in_maps, core_ids=list(range(N_CORES)))
    return _gather(res.results, n)


# revision 6
# speedup vs baseline: 4.4828x; 1.3860x over previous
"""Trainium2 Bass kernel for nn_DeletionLayer: out = where(mask, x @ W, x).

x: [200000, 1024] f32, deletion_weight: [1024, 1024] f32, mask: [200000] bool.

Sharding: data-parallel over the node axis across 8 NeuronCores. Each core
gets a uniform 25088-row shard; core 7's shard overlaps core 6's by 704 rows
(identical rows recomputed, dropped at gather) so every core runs the same
program.

Fast path (deletion_weight is a constant matrix c — true for this module,
W = ones/1000): x @ W == c * rowsum(x) broadcast across columns, so the
kernel never needs the matmul. Per 512-row supertile (bf16 I/O):
  - DMA the bf16 x supertile [128, 4, 1024].
  - DVE tensor_tensor_reduce folds the two 512-column halves and
    accumulates the f32 rowsum s per row.
  - tensor_scalar forms bias = s * (c*m) per row.
  - ACT activation computes out = (1-m)*x + bias: masked rows become c*s,
    unmasked rows are an exact bf16 passthrough.
  - DMA the bf16 supertile out.
Host converts x f32->bf16 on the way in and out bf16->f32 on the way out;
rel error vs the f32 reference is ~1e-3, well inside the 2e-2 gate.

General path (non-constant W, not exercised by the reference inputs): the
original bf16 matmul kernel (pretransposed lhsT blocks, 16 matmuls per
128-row tile accumulating into PSUM, DVE copy_predicated select, f32 I/O).
"""

from contextlib import ExitStack

import numpy as np

N_FULL = 200000
DIM = 1024
P = 128
KCH = DIM // P  # 8 contraction chunks (general path)
NCH = DIM // 512  # 2 PSUM-bank halves (general path)
R = 25088  # rows per core
T = R // P  # 196 tiles of 128 rows
N_CORES = 8
U = 14  # tiles per loop-body unroll (general path)
G = 4  # 128-row tiles per supertile (fast path)
NS = T // G  # 49 supertiles


def _build_nc_fast(ns=NS, debug=False):
    import concourse.bass as bass
    import concourse.tile as tile
    from concourse import bacc, mybir

    NS, T, R = ns, ns * G, ns * G * P  # noqa: shadowing module consts on purpose
    nc = bacc.Bacc("TRN2", target_bir_lowering=False, debug=debug)

    x_dram = nc.dram_tensor("x", [R, DIM], mybir.dt.bfloat16, kind="ExternalInput")
    cm_dram = nc.dram_tensor("cm", [P, T], mybir.dt.float32, kind="ExternalInput")
    sc_dram = nc.dram_tensor("sc", [P, T], mybir.dt.float32, kind="ExternalInput")
    o_dram = nc.dram_tensor("out", [R, DIM], mybir.dt.bfloat16, kind="ExternalOutput")

    with tile.TileContext(nc) as tc:
        with ExitStack() as ctx:
            cpool = ctx.enter_context(tc.tile_pool(name="const", bufs=1))
            xpool = ctx.enter_context(tc.tile_pool(name="x", bufs=5))
            opool = ctx.enter_context(tc.tile_pool(name="o", bufs=5))
            spool = ctx.enter_context(tc.tile_pool(name="s", bufs=4))

            cm_sb = cpool.tile([P, T], mybir.dt.float32)
            nc.sync.dma_start(cm_sb[:], cm_dram[:])
            sc_sb = cpool.tile([P, T], mybir.dt.float32)
            nc.sync.dma_start(sc_sb[:], sc_dram[:])

            for u in range(NS):
                xs = xpool.tile([P, G, DIM], mybir.dt.bfloat16, tag="xs")
                nc.sync.dma_start(
                    xs[:],
                    x_dram[bass.ts(u, G * P), :].rearrange(
                        "(g p) d -> p g d", p=P
                    ),
                )

                os = opool.tile([P, G, DIM], mybir.dt.bfloat16, tag="os")
                s = spool.tile([P, G], mybir.dt.float32, tag="s")
                bias = spool.tile([P, G], mybir.dt.float32, tag="bias")

                nc.vector.reduce_sum(s[:], xs[:], axis=mybir.AxisListType.X)
                for g in range(G):
                    nc.vector.tensor_scalar_mul(
                        bias[:, g : g + 1],
                        s[:, g : g + 1],
                        cm_sb[:, bass.ds(u * G + g, 1)],
                    )
                for g in range(G):
                    nc.scalar.activation(
                        os[:, g, :],
                        xs[:, g, :],
                        mybir.ActivationFunctionType.Identity,
                        bias=bias[:, g : g + 1],
                        scale=sc_sb[:, bass.ds(u * G + g, 1)],
                    )

                nc.scalar.dma_start(
                    o_dram[bass.ts(u, G * P), :].rearrange("(g p) d -> p g d", p=P),
                    os[:],
                )

    nc.compile()
    return nc


def _build_nc_general():
    import concourse.bass as bass
    import concourse.tile as tile
    from concourse import bacc, mybir

    n_loop = T // U
    nc = bacc.Bacc("TRN2", target_bir_lowering=False, debug=False)

    x_dram = nc.dram_tensor("x", [R, DIM], mybir.dt.float32, kind="ExternalInput")
    xt_dram = nc.dram_tensor(
        "xt", [T * DIM, P], mybir.dt.bfloat16, kind="ExternalInput"
    )
    w_dram = nc.dram_tensor("w", [DIM, DIM], mybir.dt.bfloat16, kind="ExternalInput")
    m_dram = nc.dram_tensor("mask", [P, T], mybir.dt.uint8, kind="ExternalInput")
    o_dram = nc.dram_tensor("out", [R, DIM], mybir.dt.float32, kind="ExternalOutput")

    with tile.TileContext(nc) as tc:
        with ExitStack() as ctx:
            wpool = ctx.enter_context(tc.tile_pool(name="w", bufs=1))
            xpool = ctx.enter_context(tc.tile_pool(name="x", bufs=3))
            xtpool = ctx.enter_context(tc.tile_pool(name="xt", bufs=3))
            pso_pool = ctx.enter_context(
                tc.tile_pool(name="psO", bufs=3, space="PSUM")
            )

            w_sb = wpool.tile([P, KCH, DIM], mybir.dt.bfloat16)
            nc.sync.dma_start(w_sb[:], w_dram.ap().rearrange("(c p) d -> p c d", p=P))
            m_all = wpool.tile([P, T], mybir.dt.uint8)
            nc.sync.dma_start(m_all[:], m_dram[:])

            def emit_tile(t):
                x_t = xpool.tile([P, DIM], mybir.dt.float32, tag="x")
                nc.sync.dma_start(x_t[:], x_dram[bass.ts(t, P), :])

                xT = xtpool.tile([P, KCH, P], mybir.dt.bfloat16, tag="xT")
                nc.sync.dma_start(
                    xT[:],
                    xt_dram[bass.ts(t, DIM), :].rearrange("(c i) j -> i c j", i=P),
                )

                psO = pso_pool.tile([P, DIM], mybir.dt.float32, tag="psO")
                for n in range(NCH):
                    for k in range(KCH):
                        nc.tensor.matmul(
                            psO[:, n * 512 : (n + 1) * 512],
                            xT[:, k, :],
                            w_sb[:, k, n * 512 : (n + 1) * 512],
                            start=(k == 0),
                            stop=(k == KCH - 1),
                        )

                nc.vector.copy_predicated(
                    x_t[:],
                    m_all[:, bass.ds(t, 1)].broadcast_to([P, DIM]),
                    psO[:],
                )
                nc.sync.dma_start(o_dram[bass.ts(t, P), :], x_t[:])

            with tc.For_i(0, n_loop, 1) as i:
                for j in range(U):
                    emit_tile(i * U + j)

    nc.compile()
    return nc


def _shard_starts(n):
    return [c * R for c in range(N_CORES - 1)] + [n - R]


def _core_map_fast(xs_bf, ms, c):
    # [u, g, p] row order -> per-partition tables [P, T=NS*G]
    mt = (
        ms.astype(np.float32)
        .reshape(NS, G, P)
        .transpose(2, 0, 1)
        .reshape(P, T)
    )
    return {
        "x": xs_bf,
        "cm": np.ascontiguousarray(mt * c),
        "sc": np.ascontiguousarray(1.0 - mt),
    }


def _core_map_general(xs, ms, w_bf):
    import ml_dtypes

    # xt[t, c, i, j] = x[t*128 + j, c*128 + i] — lhsT blocks, bf16
    xt = (
        np.ascontiguousarray(xs.reshape(T, P, KCH, P).transpose(0, 2, 3, 1))
        .astype(ml_dtypes.bfloat16)
        .reshape(T * DIM, P)
    )
    return {
        "x": np.ascontiguousarray(xs),
        "xt": xt,
        "w": w_bf,
        "mask": np.ascontiguousarray(ms.astype(np.uint8).reshape(T, P).T),
    }


_cached_nc_fast = None
_cached_nc_general = None


def _plan(x, deletion_weight, mask):
    """Returns (nc, in_maps, out_dtype_is_bf16)."""
    global _cached_nc_fast, _cached_nc_general
    import ml_dtypes

    n = x.shape[0]
    starts = _shard_starts(n)
    w = np.asarray(deletion_weight, dtype=np.float32)
    c = w.flat[0]
    if np.all(w == c):
        if _cached_nc_fast is None:
            _cached_nc_fast = _build_nc_fast()
        x_bf = np.asarray(x, dtype=np.float32).astype(ml_dtypes.bfloat16)
        in_maps = [
            _core_map_fast(x_bf[r0 : r0 + R], mask[r0 : r0 + R], float(c))
            for r0 in starts
        ]
        return _cached_nc_fast, in_maps, True
    if _cached_nc_general is None:
        _cached_nc_general = _build_nc_general()
    w_bf = w.astype(ml_dtypes.bfloat16)
    in_maps = [
        _core_map_general(x[r0 : r0 + R], mask[r0 : r0 + R], w_bf) for r0 in starts
    ]
    return _cached_nc_general, in_maps, False


def _gather(results, n):
    out = np.empty((n, DIM), np.float32)
    starts = _shard_starts(n)
    for c in range(N_CORES - 1):
        out[starts[c] : starts[c] + R] = results[c]["out"]
    tail = n - R * (N_CORES - 1)
    out[n - tail :] = results[-1]["out"][R - tail :]
    return out


def kernel(x, deletion_weight, mask):
    from concourse import bass_utils

    x = np.asarray(x, dtype=np.float32)
    mask = np.asarray(mask)
    n = x.shape[0]
    assert n == N_FULL and x.shape[1] == DIM

    nc, in_maps, _ = _plan(x, deletion_weight, mask)
    res = bass_utils.run_bass_kernel_spmd(nc, in_maps, core_ids=list(range(N_CORES)))
    return _gather(res.results, n)
